# revision 15
# baseline (speedup 1.0000x reference)
"""Local (windowed) attention with RoPE for Trainium2, SPMD over 8 NeuronCores.

Reference semantics (nn_LocalAttention): B,H,N,D = 4,16,4096,64, window=128,
look_backward=1, look_forward=0, pad_value=-1 (pad applies to k/v VALUES and
to the position ids; padded keys end up unmasked all -1.0 vectors).

Sharding: merged (B*H)=64 leading dim split across 8 cores, 8 slices each.
Everything else runs per-core with no collectives.
"""

import numpy as np
import ml_dtypes

import concourse.bass as bass
import concourse.bacc as bacc
import concourse.mybir as mybir
import concourse.tile as tile
from concourse.bass_utils import run_bass_kernel_spmd

F32 = mybir.dt.float32
BF16 = mybir.dt.bfloat16
NP_BF16 = ml_dtypes.bfloat16

B, H, N, D = 4, 16, 4096, 64
W = 128                    # window size
NCORES = 8
BH = B * H
BH_PER_CORE = BH // NCORES
SCALE = float(D) ** -0.5
HD = D // 2


def rope_tables(n):
    """cos/sin tables matching the reference's fp32 computation.

    sinm folds the rotate_half sign: q'[d] = q[d]*cos[d] + q[(d+32)%64]*sinm[d].
    """
    inv_freq = 1.0 / (10000.0 ** (np.arange(0, D, 2, dtype=np.float32) / np.float32(D)))
    t = np.arange(n, dtype=np.float32)
    half = t[:, None] * inv_freq[None, :]
    freqs = np.concatenate([half, half], axis=-1)  # [n, D]
    cos = np.cos(freqs).astype(np.float32)
    sin = np.sin(freqs).astype(np.float32)
    sinm = np.concatenate([-sin[:, :HD], sin[:, HD:]], axis=-1)
    return cos, sinm


def host_consts(n):
    cos, sinm = rope_tables(n)
    # tri[j, i] = 1 where key j <= query i (window-local causal keep-mask)
    j = np.arange(W)[:, None]
    i = np.arange(W)[None, :]
    tri = (j <= i).astype(NP_BF16)
    ident = np.eye(D + 1, dtype=np.float32)
    return {
        "cos_t": cos.astype(NP_BF16),
        "sinm_t": sinm.astype(NP_BF16),
        "tri": tri,
        "id65": ident,
    }


def build_nc(bh_per_core=BH_PER_CORE, n=N):
    nw = n // W
    assert nw % 2 == 0
    ns = nw // 2  # transpose slabs (2 windows each)

    nc = bacc.Bacc(None, target_bir_lowering=False)
    q_d = nc.dram_tensor("q", [bh_per_core, n, D], F32, kind="ExternalInput")
    k_d = nc.dram_tensor("k", [bh_per_core, n, D], F32, kind="ExternalInput")
    v_d = nc.dram_tensor("v", [bh_per_core, n, D], F32, kind="ExternalInput")
    cos_d = nc.dram_tensor("cos_t", [n, D], BF16, kind="ExternalInput")
    sinm_d = nc.dram_tensor("sinm_t", [n, D], BF16, kind="ExternalInput")
    tri_d = nc.dram_tensor("tri", [W, W], BF16, kind="ExternalInput")
    id_d = nc.dram_tensor("id65", [D + 1, D + 1], F32, kind="ExternalInput")
    o_d = nc.dram_tensor("out", [bh_per_core, n, D], F32, kind="ExternalOutput")

    def nat(ap):  # DRAM [n, D] -> [t, w, d] token-in-window on partitions
        return ap.rearrange("(w t) d -> t w d", t=W)

    with tile.TileContext(nc) as tc:
        with (
            tc.tile_pool(name="const", bufs=1) as constp,
            tc.tile_pool(name="io", bufs=2) as iop,
            tc.tile_pool(name="rope", bufs=2) as ropep,
            tc.tile_pool(name="stk", bufs=2) as stkp,
            tc.tile_pool(name="esb", bufs=4) as ep,
            tc.tile_pool(name="otsb", bufs=6) as otp,
            tc.tile_pool(name="rsb", bufs=3) as rp,
            tc.tile_pool(name="stage", bufs=2) as stagep,
            tc.tile_pool(name="psim", bufs=2, space="PSUM") as psimp,
            tc.tile_pool(name="pS", bufs=4, space="PSUM") as pSp,
            tc.tile_pool(name="pO", bufs=2, space="PSUM") as pOp,
        ):
            cos_sb = constp.tile([W, nw, D], BF16, tag="cos")
            nc.sync.dma_start(out=cos_sb, in_=nat(cos_d))
            sinm_sb = constp.tile([W, nw, D], BF16, tag="sinm")
            nc.sync.dma_start(out=sinm_sb, in_=nat(sinm_d))
            tri_sb = constp.tile([W, W], BF16, tag="tri")
            nc.sync.dma_start(out=tri_sb, in_=tri_d[:])
            id_sb = constp.tile([D + 1, D + 1], F32, tag="id65")
            nc.sync.dma_start(out=id_sb, in_=id_d[:])
            kpadT = constp.tile([D, W], BF16, tag="kpadT")
            nc.vector.memset(kpadT[:], -1.0)
            vpad = constp.tile([W, D + 1], BF16, tag="vpad")
            nc.vector.memset(vpad[:], -1.0)
            nc.vector.memset(vpad[:, D : D + 1], 1.0)

            for bh in range(bh_per_core):
                qn = iop.tile([W, nw, D], F32, tag="qn")
                nc.sync.dma_start(out=qn[:], in_=nat(q_d[bh]))
                kn = iop.tile([W, nw, D], F32, tag="kn")
                nc.sync.dma_start(out=kn[:], in_=nat(k_d[bh]))
                vn = iop.tile([W, nw, D], F32, tag="vn")
                nc.sync.dma_start(out=vn[:], in_=nat(v_d[bh]))

                # ---- RoPE (bf16, natural layout) ----
                # Output tiles are [W, nw, 2D] with d-columns D:2D zero -- the
                # XBAR transpose then puts every window's d-major tile at
                # partitions 0:64 (uniform matmul base partition).
                def rope(xn, tag):
                    xb = ropep.tile([W, nw, D], BF16, tag=tag + "b")
                    nc.vector.tensor_copy(out=xb[:], in_=xn[:])
                    xr = ropep.tile([W, nw, D], BF16, tag=tag + "r")
                    nc.vector.tensor_mul(
                        out=xr[:, :, 0:HD], in0=xb[:, :, HD:D], in1=sinm_sb[:, :, 0:HD]
                    )
                    nc.vector.tensor_mul(
                        out=xr[:, :, HD:D], in0=xb[:, :, 0:HD], in1=sinm_sb[:, :, HD:D]
                    )
                    xp = ropep.tile([W, nw, 2 * D], BF16, tag=tag + "p")
                    if bh < 2:  # zero the pad lanes once per pool slot
                        nc.vector.memset(xp[:, :, D : 2 * D], 0.0)
                    nc.vector.tensor_mul(out=xp[:, :, 0:D], in0=xb[:], in1=cos_sb[:])
                    nc.vector.tensor_add(
                        out=xp[:, :, 0:D], in0=xp[:, :, 0:D], in1=xr[:]
                    )
                    return xp

                qp = rope(qn, "q")
                kp = rope(kn, "k")

                # v in bf16 with a fused ones column (denominator row of S)
                vb = ropep.tile([W, nw, D + 1], BF16, tag="vb")
                nc.vector.memset(vb[:, :, D : D + 1], 1.0)
                nc.scalar.copy(out=vb[:, :, 0:D], in_=vn[:])

                # ---- d-major via XBAR dma transpose ----
                # stq[p, w, t]: p<64 -> d of window w; p>=64 -> zero pad
                stq = stkp.tile([W, nw, W], BF16, tag="stq")
                nc.sync.dma_start(
                    out=stq[:], in_=qp.rearrange("t w d -> t (w d)"), transpose=True
                )
                stk = stkp.tile([W, nw, W], BF16, tag="stk")
                nc.sync.dma_start(
                    out=stk[:], in_=kp.rearrange("t w d -> t (w d)"), transpose=True
                )

                def qT(w):  # [64, 128] moving operand for queries of window w
                    return stq[0:D, w, :]

                def kT(w):  # [64, 128] stationary operand for keys of window w
                    return stk[0:D, w, :]

                # groups of key blocks: g=0 -> (pad, 0); 1..ns-1 -> (2g-1, 2g);
                # g=ns -> (nw-1,)
                e_tiles = {}  # c -> (E tile, slot)
                o_quads = {}
                stage_sb = stagep.tile([W, nw, D], F32, tag="stage")

                def do_window(w):
                    # out^T (and denom) for window w: accumulate both key
                    # blocks' PV into one PSUM tile, evacuate, transpose.
                    et0, sl0 = e_tiles[w - 1]
                    et1, sl1 = e_tiles[w]
                    pw = pSp.tile([D + 1, W], F32, tag="s", name="pw")
                    if w == 0:
                        nc.tensor.matmul(
                            pw[:], vpad[:], et0[:, sl0, 0:W], start=True, stop=False
                        )
                    else:
                        nc.tensor.matmul(
                            pw[:], vb[:, w - 1, :], et0[:, sl0, W : 2 * W],
                            start=True, stop=False,
                        )
                    nc.tensor.matmul(
                        pw[:], vb[:, w, :], et1[:, sl1, 0:W], start=False, stop=True
                    )
                    ot = otp.tile([D + 1, W], F32, tag="ot")
                    if w % 4 == 2:  # shed some PSUM-evac load from DVE to ACT
                        nc.scalar.copy(out=ot[:], in_=pw[:])
                    else:
                        nc.vector.tensor_copy(out=ot[:], in_=pw[:])
                    qi = w // 4
                    if qi not in o_quads:
                        o_quads[qi] = pOp.tile([W, 4, D + 1], F32, tag="oq", name="oq")
                    oq = o_quads[qi]
                    sl = w % 4
                    nc.tensor.transpose(oq[:, sl, :], ot[:], id_sb[:])
                    if sl == 3 or w == nw - 1:
                        nsl = sl + 1
                        r = rp.tile([W, 4], F32, tag="r")
                        nc.vector.reciprocal(
                            out=r[:, 0:nsl], in_=oq[:, 0:nsl, D : D + 1]
                        )
                        for j in range(nsl):
                            ww = qi * 4 + j
                            nc.scalar.activation(
                                out=stage_sb[:, ww, :],
                                in_=oq[:, j, 0:D],
                                func=mybir.ActivationFunctionType.Copy,
                                scale=r[:, j : j + 1],
                            )

                for g in range(ns + 1):
                    blocks = (
                        [-1, 0] if g == 0 else ([nw - 1] if g == ns else [2 * g - 1, 2 * g])
                    )
                    simt = psimp.tile([W, 2, 2 * W], F32, tag="sim")
                    et = ep.tile([W, 2, 2 * W], BF16, tag="e")
                    for sl, c in enumerate(blocks):
                        last = c == nw - 1
                        if c == -1:
                            nc.tensor.matmul(
                                simt[:, sl, 0:W], kpadT[:], qT(0), start=True, stop=True
                            )
                        else:
                            nc.tensor.matmul(
                                simt[:, sl, 0:W], kT(c), qT(c), start=True, stop=True
                            )
                            if not last:
                                nc.tensor.matmul(
                                    simt[:, sl, W : 2 * W],
                                    kT(c),
                                    qT(c + 1),
                                    start=True,
                                    stop=True,
                                )
                    # exp (scale folded); masked entries fixed up after
                    if g == 0:
                        nc.scalar.activation(
                            out=et[:, 0, 0:W], in_=simt[:, 0, 0:W],
                            func=mybir.ActivationFunctionType.Exp, scale=SCALE,
                        )
                        nc.scalar.activation(
                            out=et[:, 1, :], in_=simt[:, 1, :],
                            func=mybir.ActivationFunctionType.Exp, scale=SCALE,
                        )
                        nc.vector.tensor_mul(
                            out=et[:, 1, 0:W], in0=et[:, 1, 0:W], in1=tri_sb[:]
                        )
                    elif g == ns:
                        nc.scalar.activation(
                            out=et[:, 0, 0:W], in_=simt[:, 0, 0:W],
                            func=mybir.ActivationFunctionType.Exp, scale=SCALE,
                        )
                        nc.vector.tensor_mul(
                            out=et[:, 0, 0:W], in0=et[:, 0, 0:W], in1=tri_sb[:]
                        )
                    else:
                        nc.scalar.activation(
                            out=et[:, :, :], in_=simt[:, :, :],
                            func=mybir.ActivationFunctionType.Exp, scale=SCALE,
                        )
                        for sl in range(2):
                            nc.vector.tensor_mul(
                                out=et[:, sl, 0:W], in0=et[:, sl, 0:W], in1=tri_sb[:]
                            )
                    for sl, c in enumerate(blocks):
                        e_tiles[c] = (et, sl)
                    # windows ready after this group
                    for w in ([0] if g == 0 else ([nw - 1] if g == ns else [2 * g - 1, 2 * g])):
                        do_window(w)
                        e_tiles.pop(w - 1, None)

                nc.sync.dma_start(out=nat(o_d[bh]), in_=stage_sb[:])

    nc.finalize()
    return nc


_built = {}
TRACE = False
LAST_RESULT = None


def _get_nc(bh_per_core=BH_PER_CORE, n=N):
    key = (bh_per_core, n)
    if key not in _built:
        _built[key] = build_nc(bh_per_core, n)
    return _built[key]


def kernel(q, k, v):
    assert q.shape == (B, H, N, D)
    qf = np.ascontiguousarray(q, dtype=np.float32).reshape(BH, N, D)
    kf = np.ascontiguousarray(k, dtype=np.float32).reshape(BH, N, D)
    vf = np.ascontiguousarray(v, dtype=np.float32).reshape(BH, N, D)
    consts = host_consts(N)
    nc = _get_nc()
    in_maps = []
    for c in range(NCORES):
        s = slice(c * BH_PER_CORE, (c + 1) * BH_PER_CORE)
        in_maps.append({"q": qf[s], "k": kf[s], "v": vf[s], **consts})
    global LAST_RESULT
    res = run_bass_kernel_spmd(nc, in_maps, list(range(NCORES)), trace=TRACE)
    LAST_RESULT = res
    out = np.concatenate([res.results[i]["out"] for i in range(NCORES)], axis=0)
    return out.reshape(B, H, N, D).astype(np.float32)
